# revision 15
# baseline (speedup 1.0000x reference)
"""Sparse attention (B=4,H=16,N=2048,D=64) on 8 trn2 NeuronCores.

Sharding: core c = bp*4 + hq handles batches [2bp, 2bp+1] x heads [4hq..4hq+3].
Per (b,h):  O^T = normalize( V~^T @ (P^T) ),  V~ = [V | 1]  (denominator for free)
  batch 0:  P^T = mask^T * exp(K Q^T/8 + bias^T)   (bias via identity-matmul in PSUM)
  batch 1:  P^T = (mask^T * exp(bias^T)) * exp(K Q^T/8)   (host-folded, streamed)
Adjacent key-tiles' S-matmuls run concurrently in PE row-groups 0-63/64-127
(q/k replicated in both partition halves; tile_position=(64*(kt%2), 0)).
Host does layout transforms and the final gather.
"""

import numpy as np
import ml_dtypes

import concourse.bass as bass
from concourse import bacc
import concourse.mybir as mybir
import concourse.tile as tile
from concourse.bass_utils import run_bass_kernel_spmd

dt = mybir.dt
AF = mybir.ActivationFunctionType

B, H, N, D = 4, 16, 2048, 64
NB = 2   # batches per core
NH = 4   # heads per core
P = 128
NKT = N // P          # 16 key tiles
QW = 512              # query tile width (one PSUM bank of fp32)
SUPW = 1024           # S supertile width (2 banks) -> fewer/larger ACT ops
NQP = N // SUPW       # 2 query supertiles
NQI = SUPW // QW      # 2 PSUM-bank columns per supertile
TRACE = False

_CACHE = {}


def build_bass():
    nc = bacc.Bacc()
    # q/k replicated in both partition halves: [NB, NH, 2D, N]
    qT = nc.declare_dram_parameter("qT", [NB, NH, 2 * D, N], dt.float16, isOutput=False)
    kT = nc.declare_dram_parameter("kT", [NB, NH, 2 * D, N], dt.float16, isOutput=False)
    vA = nc.declare_dram_parameter("vA", [NB, NH, N, D + 1], dt.float16, isOutput=False)
    maskT = nc.declare_dram_parameter("maskT", [1, N, N], dt.float16, isOutput=False)
    mex1 = nc.declare_dram_parameter("mex1", [NH, N, N], dt.float16, isOutput=False)
    biasT = nc.declare_dram_parameter("biasT", [NH, N, N], dt.float16, isOutput=False)
    ident = nc.declare_dram_parameter("ident", [P, P], dt.float16, isOutput=False)
    ones = nc.declare_dram_parameter("ones", [1, D], dt.float16, isOutput=False)
    outT = nc.declare_dram_parameter("outT", [NB, NH, D, N], dt.float32, isOutput=True)

    with tile.TileContext(nc) as tc:
        with (
            tc.tile_pool(name="const", bufs=1) as cpool,
            tc.tile_pool(name="mask", bufs=1) as mpool,
            tc.tile_pool(name="qk", bufs=2) as qkpool,
            tc.tile_pool(name="vp", bufs=2) as vpool,
            tc.tile_pool(name="bias", bufs=4) as bpool,
            tc.tile_pool(name="pt", bufs=4) as ppool,
            tc.tile_pool(name="norm", bufs=1) as rpool,
            tc.tile_pool(name="out", bufs=3) as opool_sb,
            tc.tile_pool(name="spsum", bufs=2, space="PSUM") as spool,
            tc.tile_pool(name="opsum", bufs=2, space="PSUM") as opool,
        ):
            ident_sb = cpool.tile([P, P], dt.float16, tag="ident")
            nc.sync.dma_start(ident_sb, ident[:])
            ones_sb = cpool.tile([1, D], dt.float16, tag="ones")
            nc.sync.dma_start(ones_sb, ones[:])

            # resident transposed mask for batch 0, chunks loaded at first use
            mask0 = mpool.tile([P, NKT, N], dt.float16, tag="mask0")

            for h in range(NH):
                qsb, ksb, vsb = [], [], []
                for b in range(NB):
                    qt_ = qkpool.tile([2 * D, N], dt.float16, tag=f"q{b}")
                    nc.sync.dma_start(qt_, qT[b, h])
                    kt_ = qkpool.tile([2 * D, N], dt.float16, tag=f"k{b}")
                    nc.sync.dma_start(kt_, kT[b, h])
                    vt_ = vpool.tile([P, NKT, D + 1], dt.float16, tag=f"v{b}")
                    nc.sync.dma_start(vt_, vA[b, h].rearrange("(t p) c -> p t c", p=P))
                    qsb.append(qt_)
                    ksb.append(kt_)
                    vsb.append(vt_)

                for b in range(NB):
                    for qp in range(NQP):
                        q0 = qp * SUPW
                        opsum = []
                        for qi in range(NQI):
                            ot = opool.tile([D + 1, QW], dt.float32,
                                            tag=f"o{qi}", name=f"opsum{qi}")
                            opsum.append(ot)
                        for kt0 in range(0, NKT, 2):
                            ssups = []
                            pts = []
                            for j in range(2):
                                kt = kt0 + j
                                rg = (kt % 2) * D
                                src = biasT if b == 0 else mex1
                                bias_sb = bpool.tile([P, SUPW], dt.float16,
                                                     tag="bias", name=f"bias{j}")
                                nc.sync.dma_start(
                                    bias_sb,
                                    src[h, kt * P:(kt + 1) * P, q0:q0 + SUPW],
                                )
                                if h == 0 and b == 0 and qp == 0:
                                    nc.sync.dma_start(
                                        mask0[:, kt],
                                        maskT[0, kt * P:(kt + 1) * P],
                                    )
                                ssup = spool.tile([P, SUPW], dt.float32,
                                                  tag="s", name=f"ssup{j}")
                                ssups.append((kt, rg, bias_sb, ssup))
                            # adjacent kt S-matmuls -> alternating row groups,
                            # issued pairwise for concurrency
                            for qi in range(NQI):
                                for kt, rg, _, ssup in ssups:
                                    nc.tensor.matmul(
                                        ssup[:, qi * QW:(qi + 1) * QW],
                                        ksb[b][rg:rg + D, kt * P:(kt + 1) * P],
                                        qsb[b][rg:rg + D,
                                               q0 + qi * QW:q0 + (qi + 1) * QW],
                                        start=True, stop=(b == 1),
                                        tile_position=(rg, 0),
                                    )
                            if b == 0:
                                for kt, rg, bias_sb, ssup in ssups:
                                    for qi in range(NQI):
                                        nc.tensor.matmul(
                                            ssup[:, qi * QW:(qi + 1) * QW],
                                            ident_sb,
                                            bias_sb[:, qi * QW:(qi + 1) * QW],
                                            start=False, stop=True,
                                        )
                            for kt, rg, bias_sb, ssup in ssups:
                                pt = ppool.tile([P, SUPW], dt.float16, tag="pt",
                                                name="pt")
                                nc.scalar.activation(pt, ssup, AF.Exp)
                                nc.vector.tensor_mul(
                                    pt, pt,
                                    mask0[:, kt, q0:q0 + SUPW] if b == 0
                                    else bias_sb,
                                )
                                pts.append((kt, pt))
                            for kt, pt in pts:
                                for qi in range(NQI):
                                    nc.tensor.matmul(
                                        opsum[qi],
                                        vsb[b][:, kt, :],
                                        pt[:, qi * QW:(qi + 1) * QW],
                                        start=(kt == 0), stop=(kt == NKT - 1),
                                    )
                        # normalize the two query columns of this (b, qp)
                        sums = rpool.tile([1, NQI * QW], dt.float32, tag="sums")
                        for qi in range(NQI):
                            nc.vector.tensor_copy(
                                sums[:, qi * QW:(qi + 1) * QW],
                                opsum[qi][D:D + 1, :],
                            )
                        rec = rpool.tile([1, NQI * QW], dt.float32, tag="rec")
                        nc.vector.reciprocal_approx_fast(rec, sums)
                        rec16 = rpool.tile([1, NQI * QW], dt.float16, tag="rec16")
                        nc.vector.tensor_copy(rec16, rec)
                        for qi in range(NQI):
                            bc = spool.tile([D, QW], dt.float32, tag="s")
                            nc.tensor.matmul(
                                bc, ones_sb, rec16[:, qi * QW:(qi + 1) * QW],
                                start=True, stop=True,
                            )
                            rec64 = rpool.tile([D, QW], dt.float32, tag="rec64")
                            nc.vector.tensor_copy(rec64, bc)
                            osb = opool_sb.tile([D, QW], dt.float32, tag="osb")
                            nc.vector.tensor_mul(osb, opsum[qi][:D, :], rec64)
                            nc.sync.dma_start(
                                outT[b, h, :, q0 + qi * QW:q0 + (qi + 1) * QW],
                                osb,
                            )
    nc.finalize()
    return nc


def make_in_maps(q, k, v, mask, attn_bias):
    scale = np.float32(D ** -0.5)
    qTf = (q.transpose(0, 1, 3, 2) * scale).astype(np.float16)   # [B,H,D,N]
    kTf = k.transpose(0, 1, 3, 2).astype(np.float16)
    vA = np.concatenate(
        [v, np.ones((B, H, N, 1), np.float32)], axis=-1
    ).astype(np.float16)                                         # [B,H,N,D+1]
    maskT = np.ascontiguousarray(
        mask[:, 0].transpose(0, 2, 1)
    ).astype(np.float16)                                         # [B,N,N] 0/1
    biasT32 = np.ascontiguousarray(
        attn_bias[0].transpose(0, 2, 1)
    )                                                            # [H,N,N] f32
    biasT = biasT32.astype(np.float16)
    expbT = np.exp(biasT32, dtype=np.float32).astype(np.float16)  # [H,N,N]
    ident = np.eye(P, dtype=np.float16)
    ones = np.ones((1, D), np.float16)

    in_maps = []
    for c in range(8):
        bp, hq = divmod(c, 4)
        bs, hs = 2 * bp, 4 * hq
        # replicate each batch's q/k into both partition halves: [NB,NH,2D,N]
        qrep = np.concatenate([qTf[bs:bs + NB, hs:hs + NH]] * 2, axis=2)
        krep = np.concatenate([kTf[bs:bs + NB, hs:hs + NH]] * 2, axis=2)
        in_maps.append({
            "qT": np.ascontiguousarray(qrep),
            "kT": np.ascontiguousarray(krep),
            "vA": np.ascontiguousarray(vA[bs:bs + NB, hs:hs + NH]),
            "maskT": np.ascontiguousarray(maskT[bs:bs + 1]),
            "mex1": np.ascontiguousarray(
                maskT[bs + 1][None] * expbT[hs:hs + NH]),
            "biasT": np.ascontiguousarray(biasT[hs:hs + NH]),
            "ident": ident,
            "ones": ones,
        })
    return in_maps


def kernel(q, k, v, mask, attn_bias):
    if "nc" not in _CACHE:
        _CACHE["nc"] = build_bass()
    nc = _CACHE["nc"]
    in_maps = make_in_maps(
        np.asarray(q, np.float32), np.asarray(k, np.float32),
        np.asarray(v, np.float32), np.asarray(mask, bool),
        np.asarray(attn_bias, np.float32),
    )
    rr = run_bass_kernel_spmd(
        nc, in_maps, list(range(8)), trace=TRACE,
        tmpdir=_CACHE.get("tmpdir"),
    )
    _CACHE["last_result"] = rr

    out = np.empty((B, H, N, D), np.float32)
    for c in range(8):
        bp, hq = divmod(c, 4)
        bs, hs = 2 * bp, 4 * hq
        oT = np.asarray(rr.results[c]["outT"])    # [NB,NH,D,N]
        out[bs:bs + NB, hs:hs + NH] = oT.transpose(0, 1, 3, 2)
    return out


# revision 16
# speedup vs baseline: 1.0943x; 1.0943x over previous
"""Sparse attention (B=4,H=16,N=2048,D=64) on 8 trn2 NeuronCores.

Sharding: core c = bp*4 + hq handles batches [2bp, 2bp+1] x heads [4hq..4hq+3].
Per (b,h):  O^T = normalize( V~^T @ (P^T) ),  V~ = [V | 1]  (denominator for free)
  batch 0:  P^T = mask^T * exp(K Q^T/8 + bias^T)   (bias via identity-matmul in PSUM)
  batch 1:  P^T = (mask^T * exp(bias^T)) * exp(K Q^T/8)   (host-folded, streamed)
Adjacent key-tiles' S-matmuls run concurrently in PE row-groups 0-63/64-127
(q/k replicated in both partition halves; tile_position=(64*(kt%2), 0)).
Host does layout transforms and the final gather.
"""

import numpy as np
import ml_dtypes

import concourse.bass as bass
from concourse import bacc
import concourse.mybir as mybir
import concourse.tile as tile
from concourse.bass_utils import run_bass_kernel_spmd

dt = mybir.dt
AF = mybir.ActivationFunctionType

B, H, N, D = 4, 16, 2048, 64
NB = 2   # batches per core
NH = 4   # heads per core
P = 128
NKT = N // P          # 16 key tiles
QW = 512              # query tile width (one PSUM bank of fp32)
SUPW = 1024           # S supertile width (2 banks) -> fewer/larger ACT ops
NQP = N // SUPW       # 2 query supertiles
NQI = SUPW // QW      # 2 PSUM-bank columns per supertile
TRACE = False

_CACHE = {}


def build_bass():
    nc = bacc.Bacc()
    # q/k replicated in both partition halves: [NB, NH, 2D, N]
    qT = nc.declare_dram_parameter("qT", [NB, NH, 2 * D, N], dt.float16, isOutput=False)
    kT = nc.declare_dram_parameter("kT", [NB, NH, 2 * D, N], dt.float16, isOutput=False)
    vA = nc.declare_dram_parameter("vA", [NB, NH, N, D + 1], dt.float16, isOutput=False)
    maskT = nc.declare_dram_parameter("maskT", [1, N, N], dt.float16, isOutput=False)
    mex1 = nc.declare_dram_parameter("mex1", [NH, N, N], dt.float16, isOutput=False)
    biasT = nc.declare_dram_parameter("biasT", [NH, N, N], dt.float16, isOutput=False)
    ident = nc.declare_dram_parameter("ident", [P, P], dt.float16, isOutput=False)
    ones = nc.declare_dram_parameter("ones", [1, D], dt.float16, isOutput=False)
    outT = nc.declare_dram_parameter("outT", [NB, NH, D, N], dt.float32, isOutput=True)

    with tile.TileContext(nc) as tc:
        with (
            tc.tile_pool(name="const", bufs=1) as cpool,
            tc.tile_pool(name="mask", bufs=1) as mpool,
            tc.tile_pool(name="qk", bufs=2) as qkpool,
            tc.tile_pool(name="vp", bufs=2) as vpool,
            tc.tile_pool(name="bias", bufs=4) as bpool,
            tc.tile_pool(name="pt", bufs=4) as ppool,
            tc.tile_pool(name="norm", bufs=1) as rpool,
            tc.tile_pool(name="out", bufs=3) as opool_sb,
            tc.tile_pool(name="spsum", bufs=3, space="PSUM") as spool,
            tc.tile_pool(name="opsum", bufs=1, space="PSUM") as opool,
        ):
            ident_sb = cpool.tile([P, P], dt.float16, tag="ident")
            nc.sync.dma_start(ident_sb, ident[:])
            ones_sb = cpool.tile([1, D], dt.float16, tag="ones")
            nc.sync.dma_start(ones_sb, ones[:])

            # resident transposed mask for batch 0, chunks loaded at first use
            mask0 = mpool.tile([P, NKT, N], dt.float16, tag="mask0")

            for h in range(NH):
                qsb, ksb, vsb = [], [], []
                for b in range(NB):
                    qt_ = qkpool.tile([2 * D, N], dt.float16, tag=f"q{b}")
                    nc.sync.dma_start(qt_, qT[b, h])
                    kt_ = qkpool.tile([2 * D, N], dt.float16, tag=f"k{b}")
                    nc.sync.dma_start(kt_, kT[b, h])
                    vt_ = vpool.tile([P, NKT, D + 1], dt.float16, tag=f"v{b}")
                    nc.sync.dma_start(vt_, vA[b, h].rearrange("(t p) c -> p t c", p=P))
                    qsb.append(qt_)
                    ksb.append(kt_)
                    vsb.append(vt_)

                for b in range(NB):
                    for qp in range(NQP):
                        q0 = qp * SUPW
                        opsum = []
                        for qi in range(NQI):
                            ot = opool.tile([D + 1, QW], dt.float32,
                                            tag=f"o{qi}", name=f"opsum{qi}")
                            opsum.append(ot)
                        for kt0 in range(0, NKT, 2):
                            ssups = []
                            pts = []
                            for j in range(2):
                                kt = kt0 + j
                                rg = (kt % 2) * D
                                src = biasT if b == 0 else mex1
                                bias_sb = bpool.tile([P, SUPW], dt.float16,
                                                     tag="bias", name=f"bias{j}")
                                nc.sync.dma_start(
                                    bias_sb,
                                    src[h, kt * P:(kt + 1) * P, q0:q0 + SUPW],
                                )
                                if h == 0 and b == 0 and qp == 0:
                                    nc.sync.dma_start(
                                        mask0[:, kt],
                                        maskT[0, kt * P:(kt + 1) * P],
                                    )
                                ssup = spool.tile([P, SUPW], dt.float32,
                                                  tag="s", name=f"ssup{j}")
                                ssups.append((kt, rg, bias_sb, ssup))
                            # adjacent kt S-matmuls -> alternating row groups,
                            # issued pairwise for concurrency
                            for qi in range(NQI):
                                for kt, rg, _, ssup in ssups:
                                    nc.tensor.matmul(
                                        ssup[:, qi * QW:(qi + 1) * QW],
                                        ksb[b][rg:rg + D, kt * P:(kt + 1) * P],
                                        qsb[b][rg:rg + D,
                                               q0 + qi * QW:q0 + (qi + 1) * QW],
                                        start=True, stop=(b == 1),
                                        tile_position=(rg, 0),
                                    )
                            if b == 0:
                                for kt, rg, bias_sb, ssup in ssups:
                                    for qi in range(NQI):
                                        nc.tensor.matmul(
                                            ssup[:, qi * QW:(qi + 1) * QW],
                                            ident_sb,
                                            bias_sb[:, qi * QW:(qi + 1) * QW],
                                            start=False, stop=True,
                                        )
                            for kt, rg, bias_sb, ssup in ssups:
                                pt = ppool.tile([P, SUPW], dt.float16, tag="pt",
                                                name="pt")
                                nc.scalar.activation(pt, ssup, AF.Exp)
                                nc.vector.tensor_mul(
                                    pt, pt,
                                    mask0[:, kt, q0:q0 + SUPW] if b == 0
                                    else bias_sb,
                                )
                                pts.append((kt, pt))
                            for kt, pt in pts:
                                for qi in range(NQI):
                                    nc.tensor.matmul(
                                        opsum[qi],
                                        vsb[b][:, kt, :],
                                        pt[:, qi * QW:(qi + 1) * QW],
                                        start=(kt == 0), stop=(kt == NKT - 1),
                                    )
                        # drain O accumulators to SBUF fast (frees PSUM banks
                        # while the next tile's S-phase runs), normalize there
                        otmp = []
                        for qi in range(NQI):
                            ot_sb = opool_sb.tile([D + 1, QW], dt.float32,
                                                  tag="otmp", name=f"otmp{qi}")
                            nc.scalar.copy(ot_sb, opsum[qi])
                            otmp.append(ot_sb)
                        sums = rpool.tile([1, NQI * QW], dt.float32, tag="sums")
                        for qi in range(NQI):
                            nc.vector.tensor_copy(
                                sums[:, qi * QW:(qi + 1) * QW],
                                otmp[qi][D:D + 1, :],
                            )
                        rec = rpool.tile([1, NQI * QW], dt.float32, tag="rec")
                        nc.vector.reciprocal_approx_fast(rec, sums)
                        rec16 = rpool.tile([1, NQI * QW], dt.float16, tag="rec16")
                        nc.vector.tensor_copy(rec16, rec)
                        for qi in range(NQI):
                            bc = spool.tile([D, QW], dt.float32, tag="s")
                            nc.tensor.matmul(
                                bc, ones_sb, rec16[:, qi * QW:(qi + 1) * QW],
                                start=True, stop=True,
                            )
                            rec64 = rpool.tile([D, QW], dt.float32, tag="rec64")
                            nc.vector.tensor_copy(rec64, bc)
                            osb = opool_sb.tile([D, QW], dt.float32, tag="osb")
                            nc.vector.tensor_mul(osb, otmp[qi][:D, :], rec64)
                            nc.sync.dma_start(
                                outT[b, h, :, q0 + qi * QW:q0 + (qi + 1) * QW],
                                osb,
                            )
    nc.finalize()
    return nc


def make_in_maps(q, k, v, mask, attn_bias):
    scale = np.float32(D ** -0.5)
    qTf = (q.transpose(0, 1, 3, 2) * scale).astype(np.float16)   # [B,H,D,N]
    kTf = k.transpose(0, 1, 3, 2).astype(np.float16)
    vA = np.concatenate(
        [v, np.ones((B, H, N, 1), np.float32)], axis=-1
    ).astype(np.float16)                                         # [B,H,N,D+1]
    maskT = np.ascontiguousarray(
        mask[:, 0].transpose(0, 2, 1)
    ).astype(np.float16)                                         # [B,N,N] 0/1
    biasT32 = np.ascontiguousarray(
        attn_bias[0].transpose(0, 2, 1)
    )                                                            # [H,N,N] f32
    biasT = biasT32.astype(np.float16)
    expbT = np.exp(biasT32, dtype=np.float32).astype(np.float16)  # [H,N,N]
    ident = np.eye(P, dtype=np.float16)
    ones = np.ones((1, D), np.float16)

    in_maps = []
    for c in range(8):
        bp, hq = divmod(c, 4)
        bs, hs = 2 * bp, 4 * hq
        # replicate each batch's q/k into both partition halves: [NB,NH,2D,N]
        qrep = np.concatenate([qTf[bs:bs + NB, hs:hs + NH]] * 2, axis=2)
        krep = np.concatenate([kTf[bs:bs + NB, hs:hs + NH]] * 2, axis=2)
        in_maps.append({
            "qT": np.ascontiguousarray(qrep),
            "kT": np.ascontiguousarray(krep),
            "vA": np.ascontiguousarray(vA[bs:bs + NB, hs:hs + NH]),
            "maskT": np.ascontiguousarray(maskT[bs:bs + 1]),
            "mex1": np.ascontiguousarray(
                maskT[bs + 1][None] * expbT[hs:hs + NH]),
            "biasT": np.ascontiguousarray(biasT[hs:hs + NH]),
            "ident": ident,
            "ones": ones,
        })
    return in_maps


def kernel(q, k, v, mask, attn_bias):
    if "nc" not in _CACHE:
        _CACHE["nc"] = build_bass()
    nc = _CACHE["nc"]
    in_maps = make_in_maps(
        np.asarray(q, np.float32), np.asarray(k, np.float32),
        np.asarray(v, np.float32), np.asarray(mask, bool),
        np.asarray(attn_bias, np.float32),
    )
    rr = run_bass_kernel_spmd(
        nc, in_maps, list(range(8)), trace=TRACE,
        tmpdir=_CACHE.get("tmpdir"),
    )
    _CACHE["last_result"] = rr

    out = np.empty((B, H, N, D), np.float32)
    for c in range(8):
        bp, hq = divmod(c, 4)
        bs, hs = 2 * bp, 4 * hq
        oT = np.asarray(rr.results[c]["outT"])    # [NB,NH,D,N]
        out[bs:bs + NB, hs:hs + NH] = oT.transpose(0, 1, 3, 2)
    return out


# revision 18
# speedup vs baseline: 1.3275x; 1.2130x over previous
"""Sparse attention (B=4,H=16,N=2048,D=64) on 8 trn2 NeuronCores.

Sharding: core c = bp*4 + hq handles batches [2bp, 2bp+1] x heads [4hq..4hq+3].
Per (b,h):  O^T = normalize( V~^T @ (P^T) ),  V~ = [V | 1]  (denominator for free)
  batch 0:  P^T = mask^T * exp(K Q^T/8 + bias^T)   (bias via identity-matmul in PSUM)
  batch 1:  P^T = (mask^T * exp(bias^T)) * exp(K Q^T/8)   (host-folded, streamed)
Adjacent key-tiles' S-matmuls run concurrently in PE row-groups 0-63/64-127
(q/k replicated in both partition halves; tile_position=(64*(kt%2), 0)).
Host does layout transforms and the final gather.
"""

import numpy as np
import ml_dtypes

import concourse.bass as bass
from concourse import bacc
import concourse.mybir as mybir
import concourse.tile as tile
from concourse.bass_utils import run_bass_kernel_spmd

dt = mybir.dt
AF = mybir.ActivationFunctionType

B, H, N, D = 4, 16, 2048, 64
NB = 2   # batches per core
NH = 4   # heads per core
P = 128
NKT = N // P          # 16 key tiles
QW = 512              # query tile width (one PSUM bank of fp32)
SUPW = 1024           # S supertile width (2 banks) -> fewer/larger ACT ops
NQP = N // SUPW       # 2 query supertiles
NQI = SUPW // QW      # 2 PSUM-bank columns per supertile
TRACE = False

_CACHE = {}


def build_bass():
    nc = bacc.Bacc()
    # q/k replicated in both partition halves: [NB, NH, 2D, N]
    qT = nc.declare_dram_parameter("qT", [NB, NH, 2 * D, N], dt.float16, isOutput=False)
    kT = nc.declare_dram_parameter("kT", [NB, NH, 2 * D, N], dt.float16, isOutput=False)
    vA = nc.declare_dram_parameter("vA", [NB, NH, N, D + 1], dt.float16, isOutput=False)
    maskT = nc.declare_dram_parameter("maskT", [1, N, N], dt.float16, isOutput=False)
    mex1 = nc.declare_dram_parameter("mex1", [NH, N, N], dt.float16, isOutput=False)
    biasT = nc.declare_dram_parameter("biasT", [NH, N, N], dt.float16, isOutput=False)
    ident = nc.declare_dram_parameter("ident", [P, P], dt.float16, isOutput=False)
    ones = nc.declare_dram_parameter("ones", [1, D], dt.float16, isOutput=False)
    outT = nc.declare_dram_parameter("outT", [NB, NH, D, N], dt.float32, isOutput=True)

    with tile.TileContext(nc) as tc:
        with (
            tc.tile_pool(name="const", bufs=1) as cpool,
            tc.tile_pool(name="mask", bufs=1) as mpool,
            tc.tile_pool(name="qk", bufs=2) as qkpool,
            tc.tile_pool(name="vp", bufs=2) as vpool,
            tc.tile_pool(name="bias", bufs=4) as bpool,
            tc.tile_pool(name="pt", bufs=4) as ppool,
            tc.tile_pool(name="norm", bufs=1) as rpool,
            tc.tile_pool(name="out", bufs=3) as opool_sb,
            tc.tile_pool(name="spsum", bufs=3, space="PSUM") as spool,
            tc.tile_pool(name="opsum", bufs=1, space="PSUM") as opool,
        ):
            ident_sb = cpool.tile([P, P], dt.float16, tag="ident")
            nc.sync.dma_start(ident_sb, ident[:])
            ones_sb = cpool.tile([1, D], dt.float16, tag="ones")
            nc.sync.dma_start(ones_sb, ones[:])

            # resident transposed mask for batch 0, chunks loaded at first use
            mask0 = mpool.tile([P, NKT, N], dt.float16, tag="mask0")

            for h in range(NH):
                qsb, ksb, vsb = [], [], []
                for b in range(NB):
                    qt_ = qkpool.tile([2 * D, N], dt.float16, tag=f"q{b}")
                    nc.sync.dma_start(qt_, qT[b, h])
                    kt_ = qkpool.tile([2 * D, N], dt.float16, tag=f"k{b}")
                    nc.sync.dma_start(kt_, kT[b, h])
                    vt_ = vpool.tile([P, NKT, D + 1], dt.float16, tag=f"v{b}")
                    nc.sync.dma_start(vt_, vA[b, h].rearrange("(t p) c -> p t c", p=P))
                    qsb.append(qt_)
                    ksb.append(kt_)
                    vsb.append(vt_)

                for b in range(NB):
                    for qp in range(NQP):
                        q0 = qp * SUPW
                        opsum = []
                        for qi in range(NQI):
                            ot = opool.tile([D + 1, QW], dt.float32,
                                            tag=f"o{qi}", name=f"opsum{qi}")
                            opsum.append(ot)
                        for kt0 in range(0, NKT, 2):
                            ssups = []
                            pts = []
                            for j in range(2):
                                kt = kt0 + j
                                rg = (kt % 2) * D
                                src = biasT if b == 0 else mex1
                                bias_sb = bpool.tile([P, SUPW], dt.float16,
                                                     tag="bias", name=f"bias{j}")
                                nc.sync.dma_start(
                                    bias_sb,
                                    src[h, kt * P:(kt + 1) * P, q0:q0 + SUPW],
                                )
                                if h == 0 and b == 0 and qp == 0:
                                    nc.sync.dma_start(
                                        mask0[:, kt],
                                        maskT[0, kt * P:(kt + 1) * P],
                                    )
                                ssup = spool.tile([P, SUPW], dt.float32,
                                                  tag="s", name=f"ssup{j}")
                                ssups.append((kt, rg, bias_sb, ssup))
                            # adjacent kt S-matmuls -> alternating row groups,
                            # issued pairwise for concurrency
                            for qi in range(NQI):
                                for kt, rg, _, ssup in ssups:
                                    nc.tensor.matmul(
                                        ssup[:, qi * QW:(qi + 1) * QW],
                                        ksb[b][rg:rg + D, kt * P:(kt + 1) * P],
                                        qsb[b][rg:rg + D,
                                               q0 + qi * QW:q0 + (qi + 1) * QW],
                                        start=True, stop=(b == 1),
                                        tile_position=(rg, 0),
                                    )
                            if b == 0:
                                for kt, rg, bias_sb, ssup in ssups:
                                    for qi in range(NQI):
                                        nc.tensor.matmul(
                                            ssup[:, qi * QW:(qi + 1) * QW],
                                            ident_sb,
                                            bias_sb[:, qi * QW:(qi + 1) * QW],
                                            start=False, stop=True,
                                        )
                            for kt, rg, bias_sb, ssup in ssups:
                                pt = ppool.tile([P, SUPW], dt.float16, tag="pt",
                                                name="pt")
                                nc.scalar.activation(pt, ssup, AF.Exp)
                                nc.vector.tensor_mul(
                                    pt, pt,
                                    mask0[:, kt, q0:q0 + SUPW] if b == 0
                                    else bias_sb,
                                )
                                pts.append((kt, pt))
                            for kt, pt in pts:
                                for qi in range(NQI):
                                    nc.tensor.matmul(
                                        opsum[qi],
                                        vsb[b][:, kt, :],
                                        pt[:, qi * QW:(qi + 1) * QW],
                                        start=(kt == 0), stop=(kt == NKT - 1),
                                    )
                        # drain O accumulators to SBUF fast (frees PSUM banks
                        # while the next tile's S-phase runs), normalize there
                        otmp = []
                        for qi in range(NQI):
                            ot_sb = opool_sb.tile([D + 1, QW], dt.float32,
                                                  tag="otmp", name=f"otmp{qi}")
                            nc.scalar.copy(ot_sb, opsum[qi])
                            otmp.append(ot_sb)
                        sums = rpool.tile([1, NQI * QW], dt.float32, tag="sums")
                        for qi in range(NQI):
                            nc.vector.tensor_copy(
                                sums[:, qi * QW:(qi + 1) * QW],
                                otmp[qi][D:D + 1, :],
                            )
                        rec = rpool.tile([1, NQI * QW], dt.float32, tag="rec")
                        nc.vector.reciprocal_approx_fast(rec, sums)
                        rec16 = rpool.tile([1, NQI * QW], dt.float16, tag="rec16")
                        nc.vector.tensor_copy(rec16, rec)
                        rec64 = rpool.tile([D, NQI * QW], dt.float16, tag="rec64")
                        nc.gpsimd.partition_broadcast(rec64, rec16)
                        for qi in range(NQI):
                            osb = opool_sb.tile([D, QW], dt.float32, tag="osb")
                            nc.vector.tensor_mul(
                                osb, otmp[qi][:D, :],
                                rec64[:, qi * QW:(qi + 1) * QW],
                            )
                            nc.sync.dma_start(
                                outT[b, h, :, q0 + qi * QW:q0 + (qi + 1) * QW],
                                osb,
                            )
    nc.finalize()
    return nc


def make_in_maps(q, k, v, mask, attn_bias):
    scale = np.float32(D ** -0.5)
    qTf = (q.transpose(0, 1, 3, 2) * scale).astype(np.float16)   # [B,H,D,N]
    kTf = k.transpose(0, 1, 3, 2).astype(np.float16)
    vA = np.concatenate(
        [v, np.ones((B, H, N, 1), np.float32)], axis=-1
    ).astype(np.float16)                                         # [B,H,N,D+1]
    maskT = np.ascontiguousarray(
        mask[:, 0].transpose(0, 2, 1)
    ).astype(np.float16)                                         # [B,N,N] 0/1
    biasT32 = np.ascontiguousarray(
        attn_bias[0].transpose(0, 2, 1)
    )                                                            # [H,N,N] f32
    biasT = biasT32.astype(np.float16)
    expbT = np.exp(biasT32, dtype=np.float32).astype(np.float16)  # [H,N,N]
    ident = np.eye(P, dtype=np.float16)
    ones = np.ones((1, D), np.float16)

    in_maps = []
    for c in range(8):
        bp, hq = divmod(c, 4)
        bs, hs = 2 * bp, 4 * hq
        # replicate each batch's q/k into both partition halves: [NB,NH,2D,N]
        qrep = np.concatenate([qTf[bs:bs + NB, hs:hs + NH]] * 2, axis=2)
        krep = np.concatenate([kTf[bs:bs + NB, hs:hs + NH]] * 2, axis=2)
        in_maps.append({
            "qT": np.ascontiguousarray(qrep),
            "kT": np.ascontiguousarray(krep),
            "vA": np.ascontiguousarray(vA[bs:bs + NB, hs:hs + NH]),
            "maskT": np.ascontiguousarray(maskT[bs:bs + 1]),
            "mex1": np.ascontiguousarray(
                maskT[bs + 1][None] * expbT[hs:hs + NH]),
            "biasT": np.ascontiguousarray(biasT[hs:hs + NH]),
            "ident": ident,
            "ones": ones,
        })
    return in_maps


def kernel(q, k, v, mask, attn_bias):
    if "nc" not in _CACHE:
        _CACHE["nc"] = build_bass()
    nc = _CACHE["nc"]
    in_maps = make_in_maps(
        np.asarray(q, np.float32), np.asarray(k, np.float32),
        np.asarray(v, np.float32), np.asarray(mask, bool),
        np.asarray(attn_bias, np.float32),
    )
    rr = run_bass_kernel_spmd(
        nc, in_maps, list(range(8)), trace=TRACE,
        tmpdir=_CACHE.get("tmpdir"),
    )
    _CACHE["last_result"] = rr

    out = np.empty((B, H, N, D), np.float32)
    for c in range(8):
        bp, hq = divmod(c, 4)
        bs, hs = 2 * bp, 4 * hq
        oT = np.asarray(rr.results[c]["outT"])    # [NB,NH,D,N]
        out[bs:bs + NB, hs:hs + NH] = oT.transpose(0, 1, 3, 2)
    return out
